# revision 1
# baseline (speedup 1.0000x reference)
"""Trainium2 Bass kernel for nn_DeepNet (dense MLP with BatchNorm over batch).

Reference computation (N=8192 rows, K=2 in/out features, H=4096 hidden, D=3):
    X = relu(X_in @ W_first + b_first)                      # [N, H]
    for i in range(3):
        Xh = relu(X @ W_h[i] + b_h[i])                      # [N, H]
        sq = rowwise_sum(Xh * Xh)                           # [N, 1]
        X  = bn(sq + Xh + X)        # batch stats over N, per hidden unit
    out = bn(X @ W_last + b_last + X_in)                    # [N, 2]

Strategy: data-parallel over N across 8 NeuronCores (1024 rows/core).
Activations live in SBUF in *transposed* layout A[h, m] = X[m, h] so that
  - hidden matmuls use weights as the stationary operand in natural [K, N]
    layout (no transposes anywhere),
  - BatchNorm's per-hidden-unit batch stats are free-axis reductions.
Matmuls run as float32r (full fp32 storage; PE truncates to ~13-bit mantissa,
4x faster than true fp32; end-to-end error ~4e-5).
BatchNorm uses a Welford-style merge: each core computes its local per-unit
(mean, mean^2, centered-M2); one 48KB AllReduce per BN layer combines them:
  var = sum_s M2_s / N + mean_s(mu_s^2) - mu^2   (8 equal shards of 1024)
which avoids the catastrophic E[x^2]-mu^2 cancellation (values ~2048, std ~50).
"""

import numpy as np

N_CORES = 8
N = 8192
NLOC = N // N_CORES  # 1024 rows per core
KIO = 2
H = 4096
HT = H // 128  # 32 hidden-dim tiles
D = 3
MC = 512  # matmul moving-operand chunk (fp32 max)
EPS = 1e-5

_CACHE = {}


def _build():
    import concourse.bass as bass  # noqa: F401  (registers engines)
    import concourse.mybir as mybir
    import concourse.tile as tile
    from concourse import bacc

    F32 = mybir.dt.float32
    F32R = mybir.dt.float32r
    BF16 = mybir.dt.bfloat16
    ALU = mybir.AluOpType
    ACTF = mybir.ActivationFunctionType
    AX = mybir.AxisListType.X

    nc = bacc.Bacc("TRN2", target_bir_lowering=False, debug=False, num_devices=N_CORES)

    xin = nc.dram_tensor("x_in", [NLOC, KIO], F32R, kind="ExternalInput")
    wf = nc.dram_tensor("w_first", [KIO, H], F32R, kind="ExternalInput")
    bf = nc.dram_tensor("b_first", [H], F32, kind="ExternalInput")
    wh = nc.dram_tensor("w_h", [D, H, H], F32R, kind="ExternalInput")
    bh = nc.dram_tensor("b_h", [D, H], F32, kind="ExternalInput")
    wl = nc.dram_tensor("w_last", [H, KIO], F32R, kind="ExternalInput")
    bl = nc.dram_tensor("b_last", [KIO], F32, kind="ExternalInput")
    yx = nc.dram_tensor("y", [NLOC, KIO], F32, kind="ExternalOutput")
    ah_spill = nc.dram_tensor("ah_spill", [H, NLOC], BF16)

    groups = [list(range(N_CORES))]

    def mm_cast(ap):
        return ap

    with tile.TileContext(nc) as tc:
        with (
            tc.tile_pool(name="a", bufs=1) as apool,
            tc.tile_pool(name="w", bufs=2) as wpool,
            tc.tile_pool(name="ahw", bufs=2) as ahw_pool,
            tc.tile_pool(name="ypw", bufs=2) as ypw_pool,
            tc.tile_pool(name="ahr", bufs=3) as ahr_pool,
            tc.tile_pool(name="sc", bufs=2) as sc_pool,
            tc.tile_pool(name="sqw", bufs=1) as sqw_pool,
            tc.tile_pool(name="st", bufs=1) as st_pool,
            tc.tile_pool(name="st2", bufs=2) as st2_pool,
            tc.tile_pool(name="ps", bufs=2, space="PSUM") as ps_pool,
            tc.tile_pool(name="sqps", bufs=1, space="PSUM") as sq_pool,
            tc.tile_pool(name="dram", bufs=1, space="DRAM") as dpool,
        ):
            A = apool.tile([128, HT * NLOC], F32R)

            ones_f = st_pool.tile([128, 1], F32)
            nc.vector.memset(ones_f[:], 1.0)
            ones_t = st_pool.tile([128, 1], F32R)
            nc.vector.tensor_copy(ones_t[:], ones_f[:])
            eps_t = st_pool.tile([128, 1], F32)
            nc.vector.memset(eps_t[:], EPS)
            xtr = st_pool.tile([KIO, NLOC], F32R)
            nc.sync.dma_start(xtr[:], xin.rearrange("m k -> k m"))
            bf_t = st_pool.tile([128, HT], F32)
            nc.sync.dma_start(bf_t[:], bf.rearrange("(t p) -> p t", p=128))

            suma_a = st_pool.tile([128, HT], F32)
            suma_b = st_pool.tile([128, HT], F32)
            sumA = st_pool.tile([128, HT], F32)
            m2a = st_pool.tile([128, HT], F32)
            mu_s = st_pool.tile([128, HT], F32)
            mu2 = st_pool.tile([128, HT], F32)
            tmp1 = st_pool.tile([128, HT], F32)
            tmp2 = st_pool.tile([128, HT], F32)
            var_t = st_pool.tile([128, HT], F32)
            std_t = st_pool.tile([128, HT], F32)
            scale_t = st_pool.tile([128, HT], F32)
            ds_t = st_pool.tile([128, HT], F32)
            sq_sb = st_pool.tile([1, NLOC], F32)
            ssq_bc = st_pool.tile([128, 1], F32)
            bsq = st_pool.tile([128, NLOC], F32)

            # ---------------- first layer: A = relu(W_first^T X_in^T + b) ----
            wf_t = wpool.tile([KIO, H], F32R, tag="w")
            nc.sync.dma_start(wf_t[:], wf[:, :])
            for n in range(HT):
                ps0 = ps_pool.tile([128, MC], F32, tag="ps0")
                ps1 = ps_pool.tile([128, MC], F32, tag="ps1")
                lhsT = mm_cast(wf_t[:, n * 128 : (n + 1) * 128])
                nc.tensor.matmul(ps0[:], lhsT, mm_cast(xtr[:, 0:MC]), start=True, stop=True)
                nc.tensor.matmul(ps1[:], lhsT, mm_cast(xtr[:, MC:NLOC]), start=True, stop=True)
                a_sl = A[:, n * NLOC : (n + 1) * NLOC]
                nc.scalar.activation(
                    a_sl[:, 0:MC], ps0[:], ACTF.Relu,
                    bias=bf_t[:, n : n + 1], accum_out=suma_a[:, n : n + 1],
                )
                nc.scalar.activation(
                    a_sl[:, MC:NLOC], ps1[:], ACTF.Relu,
                    bias=bf_t[:, n : n + 1], accum_out=suma_b[:, n : n + 1],
                )
            nc.vector.tensor_tensor(sumA[:], suma_a[:], suma_b[:], op=ALU.add)

            # warm up the collective rings while the PE is busy with layer 1:
            # the first AllReduce otherwise pays ~20us of cold-start inside the
            # first BN tail
            ccw_in = dpool.tile([1, 1], F32, tag="ccw_in")
            ccw_out = dpool.tile([1, 1], F32, tag="ccw_out")
            nc.gpsimd.dma_start(ccw_in[:], ones_f[0:1, 0:1])
            nc.gpsimd.collective_compute(
                "AllReduce", ALU.add, replica_groups=groups,
                ins=[ccw_in.opt()], outs=[ccw_out.opt()],
            )

            # ---------------- hidden layers ----------------
            for li in range(D):
                bh_t = st2_pool.tile([128, HT], F32, tag="bh")
                nc.sync.dma_start(bh_t[:], bh[li].rearrange("(t p) -> p t", p=128))
                sqp0 = sq_pool.tile([1, MC], F32, tag="sq0")
                sqp1 = sq_pool.tile([1, MC], F32, tag="sq1")

                # matmul phase: Ah = relu(W^T A + b), sq += ones^T Ah^2
                for n in range(HT):
                    wcol = wpool.tile([128, HT * 128], F32R, tag="w")
                    nc.sync.dma_start(
                        wcol[:].rearrange("p (t c) -> p t c", c=128),
                        wh[li, :, n * 128 : (n + 1) * 128].rearrange(
                            "(t p) c -> p t c", p=128
                        ),
                    )
                    ps0 = ps_pool.tile([128, MC], F32, tag="ps0")
                    ps1 = ps_pool.tile([128, MC], F32, tag="ps1")
                    for k in range(HT):
                        lhsT = mm_cast(wcol[:, k * 128 : (k + 1) * 128])
                        a_k = A[:, k * NLOC : (k + 1) * NLOC]
                        nc.tensor.matmul(
                            ps0[:], lhsT, mm_cast(a_k[:, 0:MC]),
                            start=(k == 0), stop=(k == HT - 1),
                        )
                        nc.tensor.matmul(
                            ps1[:], lhsT, mm_cast(a_k[:, MC:NLOC]),
                            start=(k == 0), stop=(k == HT - 1),
                        )
                    ah_t = ahw_pool.tile([128, NLOC], F32, tag="ahw")
                    nc.scalar.activation(
                        ah_t[:, 0:MC], ps0[:], ACTF.Relu,
                        bias=bh_t[:, n : n + 1], accum_out=suma_a[:, n : n + 1],
                    )
                    nc.scalar.activation(
                        ah_t[:, MC:NLOC], ps1[:], ACTF.Relu,
                        bias=bh_t[:, n : n + 1], accum_out=suma_b[:, n : n + 1],
                    )
                    ah2_0 = sc_pool.tile([128, MC], F32R, tag="ah2")
                    nc.scalar.activation(ah2_0[:], ah_t[:, 0:MC], ACTF.Square)
                    nc.tensor.matmul(
                        sqp0[:], mm_cast(ones_t[:]), mm_cast(ah2_0[:]),
                        start=(n == 0), stop=(n == HT - 1),
                    )
                    ah2_1 = sc_pool.tile([128, MC], F32R, tag="ah2")
                    nc.scalar.activation(ah2_1[:], ah_t[:, MC:NLOC], ACTF.Square)
                    nc.tensor.matmul(
                        sqp1[:], mm_cast(ones_t[:]), mm_cast(ah2_1[:]),
                        start=(n == 0), stop=(n == HT - 1),
                    )
                    # fold the residual add into the matmul phase (DVE is idle
                    # here); spill Y_partial = Ah + A in bf16 (only feeds the
                    # residual path; sq/M2 stay on the f32 chain)
                    yp_t = ypw_pool.tile([128, NLOC], BF16, tag="ypw")
                    nc.vector.tensor_tensor(
                        yp_t[:], ah_t[:], A[:, n * NLOC : (n + 1) * NLOC], op=ALU.add
                    )
                    nc.sync.dma_start(ah_spill[n * 128 : (n + 1) * 128, :], yp_t[:])

                # local stats: mu_s = (sum_m Ah + sum_m A + sum_m sq) / NLOC
                nc.vector.tensor_copy(sq_sb[:, 0:MC], sqp0[:])
                nc.vector.tensor_copy(sq_sb[:, MC:NLOC], sqp1[:])
                ssq = st2_pool.tile([1, 1], F32, tag="ssq")
                nc.vector.reduce_sum(ssq[:], sq_sb[:], axis=AX)
                nc.gpsimd.partition_broadcast(ssq_bc[:], ssq[:])
                nc.gpsimd.partition_broadcast(bsq[:], sq_sb[:])
                nc.vector.tensor_tensor(tmp1[:], suma_a[:], suma_b[:], op=ALU.add)
                nc.vector.tensor_tensor(tmp1[:], tmp1[:], sumA[:], op=ALU.add)
                nc.vector.tensor_scalar(
                    mu_s[:], tmp1[:], scalar1=ssq_bc[:, 0:1], scalar2=1.0 / NLOC,
                    op0=ALU.add, op1=ALU.mult,
                )
                nc.vector.tensor_tensor(mu2[:], mu_s[:], mu_s[:], op=ALU.mult)

                # W pass: A <- (A - mu_s) + Ah + bsq  (centered pre-BN), M2 accum
                for n in range(HT):
                    a_sl = A[:, n * NLOC : (n + 1) * NLOC]
                    ahr_t = ahr_pool.tile([128, NLOC], BF16, tag="ahr")
                    nc.sync.dma_start(ahr_t[:], ah_spill[n * 128 : (n + 1) * 128, :])
                    nc.vector.scalar_tensor_tensor(
                        a_sl, bsq[:], mu_s[:, n : n + 1], ahr_t[:],
                        op0=ALU.subtract, op1=ALU.add,
                    )
                    sc0 = sqw_pool.tile([128, NLOC], F32, tag="sqw")
                    nc.scalar.activation(
                        sc0[:], a_sl, ACTF.Square,
                        accum_out=m2a[:, n : n + 1],
                    )

                # Welford all-reduce of (mu_s, mu_s^2, M2_s)
                cc_in = dpool.tile([128, 3 * HT], F32, tag="cc_in")
                cc_out = dpool.tile([128, 3 * HT], F32, tag="cc_out")
                nc.gpsimd.dma_start(cc_in[:, 0:HT], mu_s[:])
                nc.gpsimd.dma_start(cc_in[:, HT : 2 * HT], mu2[:])
                nc.gpsimd.dma_start(cc_in[:, 2 * HT : 3 * HT], m2a[:])
                nc.gpsimd.collective_compute(
                    "AllReduce", ALU.add, replica_groups=groups,
                    ins=[cc_in.opt()], outs=[cc_out.opt()],
                )
                red = st2_pool.tile([128, 3 * HT], F32, tag="red")
                nc.gpsimd.dma_start(red[:], cc_out[:])

                # mu = sum(mu_s)/8 ; var = sumM2/N + sum(mu_s^2)/8 - mu^2
                mu = tmp1
                nc.vector.tensor_scalar(
                    mu[:], red[:, 0:HT], scalar1=1.0 / N_CORES, scalar2=None,
                    op0=ALU.mult,
                )
                nc.vector.tensor_scalar(
                    var_t[:], red[:, 2 * HT : 3 * HT], scalar1=1.0 / N, scalar2=None,
                    op0=ALU.mult,
                )
                nc.vector.tensor_scalar(
                    tmp2[:], red[:, HT : 2 * HT], scalar1=1.0 / N_CORES, scalar2=None,
                    op0=ALU.mult,
                )
                nc.vector.tensor_tensor(var_t[:], var_t[:], tmp2[:], op=ALU.add)
                nc.vector.tensor_tensor(tmp2[:], mu[:], mu[:], op=ALU.mult)
                nc.vector.tensor_tensor(var_t[:], var_t[:], tmp2[:], op=ALU.subtract)
                nc.scalar.activation(std_t[:], var_t[:], ACTF.Sqrt, bias=eps_t[:, 0:1])
                nc.vector.reciprocal(scale_t[:], std_t[:])
                # delta = mu - mu_s ; ds = -delta*scale ; A <- A*scale + ds
                nc.vector.tensor_tensor(tmp2[:], mu[:], mu_s[:], op=ALU.subtract)
                nc.vector.tensor_tensor(tmp2[:], tmp2[:], scale_t[:], op=ALU.mult)
                nc.vector.tensor_scalar(
                    ds_t[:], tmp2[:], scalar1=-1.0, scalar2=None, op0=ALU.mult,
                )
                for n in range(HT):
                    a_sl = A[:, n * NLOC : (n + 1) * NLOC]
                    nc.vector.tensor_scalar(
                        a_sl, a_sl, scalar1=scale_t[:, n : n + 1],
                        scalar2=ds_t[:, n : n + 1], op0=ALU.mult, op1=ALU.add,
                    )
                # sum_m of new A per unit = NLOC * ds  (sum of centered W is 0)
                nc.vector.tensor_scalar(
                    sumA[:], ds_t[:], scalar1=float(NLOC), scalar2=None, op0=ALU.mult,
                )

            # ---------------- last layer + final BN ----------------
            wl_t = st_pool.tile([128, HT * KIO], F32R)
            nc.sync.dma_start(
                wl_t[:].rearrange("p (t c) -> p t c", c=KIO),
                wl.rearrange("(t p) c -> p t c", p=128),
            )
            bl_t = st_pool.tile([KIO, 1], F32)
            nc.sync.dma_start(bl_t[:], bl[:].unsqueeze(1))
            psl0 = ps_pool.tile([KIO, MC], F32, tag="ps0")
            psl1 = ps_pool.tile([KIO, MC], F32, tag="ps1")
            for k in range(HT):
                lhsT = mm_cast(wl_t[:, k * KIO : (k + 1) * KIO])
                a_k = A[:, k * NLOC : (k + 1) * NLOC]
                nc.tensor.matmul(
                    psl0[:], lhsT, mm_cast(a_k[:, 0:MC]),
                    start=(k == 0), stop=(k == HT - 1),
                )
                nc.tensor.matmul(
                    psl1[:], lhsT, mm_cast(a_k[:, MC:NLOC]),
                    start=(k == 0), stop=(k == HT - 1),
                )
            yl = st_pool.tile([KIO, NLOC], F32)
            nc.vector.tensor_tensor(yl[:, 0:MC], psl0[:], xtr[:, 0:MC], op=ALU.add)
            nc.vector.tensor_tensor(yl[:, MC:NLOC], psl1[:], xtr[:, MC:NLOC], op=ALU.add)
            nc.vector.tensor_scalar(
                yl[:], yl[:], scalar1=bl_t[:, 0:1], scalar2=None, op0=ALU.add,
            )
            mu_sl = st_pool.tile([KIO, 1], F32)
            nc.vector.reduce_sum(mu_sl[:], yl[:], axis=AX)
            nc.vector.tensor_scalar(
                mu_sl[:], mu_sl[:], scalar1=1.0 / NLOC, scalar2=None, op0=ALU.mult,
            )
            nc.vector.tensor_scalar(
                yl[:], yl[:], scalar1=mu_sl[:, 0:1], scalar2=None, op0=ALU.subtract,
            )
            m2l = st_pool.tile([KIO, 1], F32)
            scr = sqw_pool.tile([KIO, NLOC], F32, tag="sqw")
            nc.scalar.activation(scr[:], yl[:], ACTF.Square, accum_out=m2l[:, 0:1])
            mu2l = st_pool.tile([KIO, 1], F32)
            nc.vector.tensor_tensor(mu2l[:], mu_sl[:], mu_sl[:], op=ALU.mult)
            cpl = st_pool.tile([KIO, 3], F32)
            nc.vector.tensor_copy(cpl[:, 0:1], mu_sl[:])
            nc.vector.tensor_copy(cpl[:, 1:2], mu2l[:])
            nc.vector.tensor_copy(cpl[:, 2:3], m2l[:])
            ccl_in = dpool.tile([KIO, 3], F32, tag="ccl_in")
            ccl_out = dpool.tile([KIO, 3], F32, tag="ccl_out")
            nc.gpsimd.dma_start(ccl_in[:], cpl[:])
            nc.gpsimd.collective_compute(
                "AllReduce", ALU.add, replica_groups=groups,
                ins=[ccl_in.opt()], outs=[ccl_out.opt()],
            )
            redl = st_pool.tile([KIO, 3], F32)
            nc.gpsimd.dma_start(redl[:], ccl_out[:])
            mul_t = st_pool.tile([KIO, 1], F32)
            nc.vector.tensor_scalar(
                mul_t[:], redl[:, 0:1], scalar1=1.0 / N_CORES, scalar2=None,
                op0=ALU.mult,
            )
            varl = st_pool.tile([KIO, 1], F32)
            tl2 = st_pool.tile([KIO, 1], F32)
            nc.vector.tensor_scalar(
                varl[:], redl[:, 2:3], scalar1=1.0 / N, scalar2=None, op0=ALU.mult,
            )
            nc.vector.tensor_scalar(
                tl2[:], redl[:, 1:2], scalar1=1.0 / N_CORES, scalar2=None, op0=ALU.mult,
            )
            nc.vector.tensor_tensor(varl[:], varl[:], tl2[:], op=ALU.add)
            nc.vector.tensor_tensor(tl2[:], mul_t[:], mul_t[:], op=ALU.mult)
            nc.vector.tensor_tensor(varl[:], varl[:], tl2[:], op=ALU.subtract)
            stdl = st_pool.tile([KIO, 1], F32)
            nc.scalar.activation(stdl[:], varl[:], ACTF.Sqrt, bias=eps_t[0:KIO, 0:1])
            scalel = st_pool.tile([KIO, 1], F32)
            nc.vector.reciprocal(scalel[:], stdl[:])
            nc.vector.tensor_tensor(tl2[:], mul_t[:], mu_sl[:], op=ALU.subtract)
            nc.vector.tensor_tensor(tl2[:], tl2[:], scalel[:], op=ALU.mult)
            dsl = st_pool.tile([KIO, 1], F32)
            nc.vector.tensor_scalar(
                dsl[:], tl2[:], scalar1=-1.0, scalar2=None, op0=ALU.mult,
            )
            nc.vector.tensor_scalar(
                yl[:], yl[:], scalar1=scalel[:, 0:1], scalar2=dsl[:, 0:1],
                op0=ALU.mult, op1=ALU.add,
            )
            nc.sync.dma_start(yx.rearrange("m k -> k m"), yl[:])

    nc.compile()
    return nc


def _get_nc():
    if "nc" not in _CACHE:
        _CACHE["nc"] = _build()
    return _CACHE["nc"]


def kernel(**inputs):
    from concourse.bass_utils import run_bass_kernel_spmd

    nc = _get_nc()
    x_in = np.ascontiguousarray(np.asarray(inputs["X_in"], dtype=np.float32))
    shared = {
        "w_first": np.ascontiguousarray(np.asarray(inputs["W_first"], np.float32)),
        "b_first": np.ascontiguousarray(np.asarray(inputs["b_first"], np.float32)),
        "w_h": np.ascontiguousarray(np.asarray(inputs["W_h"], np.float32)),
        "b_h": np.ascontiguousarray(np.asarray(inputs["b_h"], np.float32)),
        "w_last": np.ascontiguousarray(np.asarray(inputs["W_last"], np.float32)),
        "b_last": np.ascontiguousarray(np.asarray(inputs["b_last"], np.float32)),
    }
    in_maps = [
        {"x_in": x_in[c * NLOC : (c + 1) * NLOC], **shared} for c in range(N_CORES)
    ]
    res = run_bass_kernel_spmd(nc, in_maps, list(range(N_CORES)))
    out = np.concatenate([res.results[c]["y"] for c in range(N_CORES)], axis=0)
    return out.astype(np.float32)



# revision 2
# speedup vs baseline: 1.3184x; 1.3184x over previous
"""Trainium2 Bass kernel for nn_DeepNet (dense MLP with BatchNorm over batch).

Reference computation (N=8192 rows, K=2 in/out features, H=4096 hidden, D=3):
    X = relu(X_in @ W_first + b_first)                      # [N, H]
    for i in range(3):
        Xh = relu(X @ W_h[i] + b_h[i])                      # [N, H]
        sq = rowwise_sum(Xh * Xh)                           # [N, 1]
        X  = bn(sq + Xh + X)        # batch stats over N, per hidden unit
    out = bn(X @ W_last + b_last + X_in)                    # [N, 2]

Strategy: data-parallel over N across 8 NeuronCores (1024 rows/core).
Activations live in SBUF transposed: A[h, m] = X[m, h].

v2 design vs baseline:
  - Weights + activations in bf16: LDWEIGHTS drops 224->~107ns (fully hidden
    behind the 213ns N=512 matmul stream); fp32r paid 272ns/MM = LDW-bound.
  - Host-side prep: weights pre-permuted to DMA-contiguous layout + cast to
    bf16; X_in pre-transposed; output returned transposed (host transposes
    back). Kills the 9us/element-descriptor DMAs at head/tail.
  - No DRAM spill: yp = Ah + A kept in a second SBUF buffer (bf16).
  - One-pass BN stats: A <- y' = yp + (sq - mhat) (centered pre-BN, bf16);
    var via E[y'^2] with the Welford-style shift identity Q_s = E_shard[y^2]
    = E[y'^2] - d^2 + mu_s^2 (no catastrophic cancellation; only the final
    global var = mean(Q) - mu^2 subtracts big numbers, err ~0.04% of var).
  - AllReduce payload (mu_s, Q_s) = 32KB, one per hidden layer + tiny final.
  - BN affine (scale/shift) folded into the *next* layer's n=0 k-loop as an
    in-place tensor_scalar per k-tile, pipelined under the matmuls.
  - sq ones-matmuls delayed by one n-tile so PE never waits on Scalar.
"""

import numpy as np

N_CORES = 8
N = 8192
NLOC = N // N_CORES  # 1024 rows per core
KIO = 2
H = 4096
HT = H // 128  # 32 hidden-dim tiles
D = 3
MC = 512  # matmul moving-operand chunk (one PSUM bank of fp32)
EPS = 1e-5

_CACHE = {}


def _build():
    import concourse.bass as bass  # noqa: F401  (registers engines)
    import concourse.mybir as mybir
    import concourse.tile as tile
    from concourse import bacc

    F32 = mybir.dt.float32
    BF16 = mybir.dt.bfloat16
    ALU = mybir.AluOpType
    ACTF = mybir.ActivationFunctionType
    AX = mybir.AxisListType.X

    nc = bacc.Bacc("TRN2", target_bir_lowering=False, debug=False, num_devices=N_CORES)

    xt_bf_d = nc.dram_tensor("xt_bf", [KIO, NLOC], BF16, kind="ExternalInput")
    xt_f_d = nc.dram_tensor("xt_f", [KIO, NLOC], F32, kind="ExternalInput")
    wf_d = nc.dram_tensor("wf", [KIO, H], BF16, kind="ExternalInput")
    bft_d = nc.dram_tensor("bft", [128, HT], F32, kind="ExternalInput")
    whp_d = nc.dram_tensor("whp", [D, HT, 128, HT, 128], BF16, kind="ExternalInput")
    bht_d = nc.dram_tensor("bht", [D, 128, HT], F32, kind="ExternalInput")
    wlt_d = nc.dram_tensor("wlt", [128, HT * KIO], BF16, kind="ExternalInput")
    blt_d = nc.dram_tensor("blt", [KIO, 1], F32, kind="ExternalInput")
    y_d = nc.dram_tensor("y", [KIO, NLOC], F32, kind="ExternalOutput")

    groups = [list(range(N_CORES))]

    with tile.TileContext(nc) as tc:
        with (
            tc.tile_pool(name="a", bufs=1) as apool,
            tc.tile_pool(name="yp", bufs=1) as yppool,
            tc.tile_pool(name="w", bufs=2) as wpool,
            tc.tile_pool(name="ahw", bufs=3) as ahw_pool,
            tc.tile_pool(name="sc", bufs=4) as sc_pool,
            tc.tile_pool(name="scr", bufs=2) as scr_pool,
            tc.tile_pool(name="st", bufs=1) as st_pool,
            tc.tile_pool(name="st2", bufs=2) as st2_pool,
            tc.tile_pool(name="ps", bufs=2, space="PSUM") as ps_pool,
            tc.tile_pool(name="sqps", bufs=1, space="PSUM") as sq_pool,
            tc.tile_pool(name="dram", bufs=1, space="DRAM") as dpool,
        ):
            A = apool.tile([128, HT * NLOC], BF16)
            YP = yppool.tile([128, HT * NLOC], BF16)

            ones_bf = st_pool.tile([128, 1], BF16)
            nc.vector.memset(ones_bf[:], 1.0)
            eps_t = st_pool.tile([128, 1], F32)
            nc.vector.memset(eps_t[:], EPS)

            xt_bf = st_pool.tile([KIO, NLOC], BF16)
            nc.sync.dma_start(xt_bf[:], xt_bf_d[:, :])
            xt_f = st_pool.tile([KIO, NLOC], F32)
            nc.sync.dma_start(xt_f[:], xt_f_d[:, :])
            wf_t = st_pool.tile([KIO, H], BF16)
            nc.sync.dma_start(wf_t[:], wf_d[:, :])
            bf_t = st_pool.tile([128, HT], F32)
            nc.sync.dma_start(bf_t[:], bft_d[:, :])

            # per-h stat accumulators (f32)
            suma_a = st_pool.tile([128, HT], F32)
            suma_b = st_pool.tile([128, HT], F32)
            sumA = st_pool.tile([128, HT], F32)
            sumY2 = st_pool.tile([128, HT], F32)
            mu_s = st_pool.tile([128, HT], F32)
            d_t = st_pool.tile([128, HT], F32)
            q_t = st_pool.tile([128, HT], F32)
            tmp1 = st_pool.tile([128, HT], F32)
            tmp2 = st_pool.tile([128, HT], F32)
            tmp3 = st_pool.tile([128, HT], F32)
            var_t = st_pool.tile([128, HT], F32)
            std_t = st_pool.tile([128, HT], F32)
            s_t = st_pool.tile([128, HT], F32)
            c_t = st_pool.tile([128, HT], F32)
            sq_sb = st_pool.tile([1, NLOC], F32)
            sq_c = st_pool.tile([1, NLOC], F32)
            sq_cbf = st_pool.tile([1, NLOC], BF16)
            bsq_bf = st_pool.tile([128, NLOC], BF16)
            ssq = st_pool.tile([1, 1], F32)
            mhat = st_pool.tile([1, 1], F32)
            ssq_bc = st_pool.tile([128, 1], F32)
            mhat_bc = st_pool.tile([128, 1], F32)

            # ---------------- first layer: A = relu(W_first^T X^T + b) -------
            for n in range(HT):
                ps0 = ps_pool.tile([128, MC], F32, tag="ps0")
                ps1 = ps_pool.tile([128, MC], F32, tag="ps1")
                lhsT = wf_t[:, n * 128 : (n + 1) * 128]
                nc.tensor.matmul(ps0[:], lhsT, xt_bf[:, 0:MC], start=True, stop=True)
                nc.tensor.matmul(ps1[:], lhsT, xt_bf[:, MC:NLOC], start=True, stop=True)
                a_sl = A[:, n * NLOC : (n + 1) * NLOC]
                nc.scalar.activation(
                    a_sl[:, 0:MC], ps0[:], ACTF.Relu,
                    bias=bf_t[:, n : n + 1], accum_out=suma_a[:, n : n + 1],
                )
                nc.scalar.activation(
                    a_sl[:, MC:NLOC], ps1[:], ACTF.Relu,
                    bias=bf_t[:, n : n + 1], accum_out=suma_b[:, n : n + 1],
                )
            nc.vector.tensor_tensor(sumA[:], suma_a[:], suma_b[:], op=ALU.add)

            # warm up the collective path with a full-size AllReduce while the
            # PE is busy with layer 1 (cold first AR otherwise costs ~2x)
            cc_in = dpool.tile([128, 2 * HT], F32, tag="cc_in")
            cc_out = dpool.tile([128, 2 * HT], F32, tag="cc_out")
            nc.gpsimd.dma_start(cc_in[:, 0:HT], suma_a[:])
            nc.gpsimd.dma_start(cc_in[:, HT : 2 * HT], suma_b[:])
            nc.gpsimd.collective_compute(
                "AllReduce", ALU.add, replica_groups=groups,
                ins=[cc_in.opt()], outs=[cc_out.opt()],
            )

            # ---------------- hidden layers ----------------
            for li in range(D):
                bh_t = st2_pool.tile([128, HT], F32, tag="bh")
                nc.sync.dma_start(bh_t[:], bht_d[li])
                sqp0 = sq_pool.tile([1, MC], F32, tag="sq0")
                sqp1 = sq_pool.tile([1, MC], F32, tag="sq1")

                prev_sq = None
                for n in range(HT):
                    wcol = wpool.tile([128, HT * 128], BF16, tag="w")
                    nc.sync.dma_start(
                        wcol[:].rearrange("p (t c) -> p t c", c=128),
                        whp_d[li, n],
                    )
                    ps0 = ps_pool.tile([128, MC], F32, tag="ps0")
                    ps1 = ps_pool.tile([128, MC], F32, tag="ps1")
                    for k in range(HT):
                        a_k = A[:, k * NLOC : (k + 1) * NLOC]
                        if li > 0 and n == 0:
                            # fold previous layer's BN affine into this pass
                            nc.vector.tensor_scalar(
                                a_k, a_k,
                                scalar1=s_t[:, k : k + 1], scalar2=c_t[:, k : k + 1],
                                op0=ALU.mult, op1=ALU.add,
                            )
                        lhsT = wcol[:, k * 128 : (k + 1) * 128]
                        nc.tensor.matmul(
                            ps0[:], lhsT, a_k[:, 0:MC],
                            start=(k == 0), stop=(k == HT - 1),
                        )
                        nc.tensor.matmul(
                            ps1[:], lhsT, a_k[:, MC:NLOC],
                            start=(k == 0), stop=(k == HT - 1),
                        )
                    # sq matmuls for the PREVIOUS tile (gives Scalar a full
                    # k-loop of slack -> PE never waits on relu/square)
                    if prev_sq is not None:
                        pa0, pa1, pn = prev_sq
                        nc.tensor.matmul(
                            sqp0[:], ones_bf[:], pa0[:],
                            start=(pn == 0), stop=False,
                        )
                        nc.tensor.matmul(
                            sqp1[:], ones_bf[:], pa1[:],
                            start=(pn == 0), stop=False,
                        )
                    ah_t = ahw_pool.tile([128, NLOC], BF16, tag="ahw")
                    nc.scalar.activation(
                        ah_t[:, 0:MC], ps0[:], ACTF.Relu,
                        bias=bh_t[:, n : n + 1], accum_out=suma_a[:, n : n + 1],
                    )
                    nc.scalar.activation(
                        ah_t[:, MC:NLOC], ps1[:], ACTF.Relu,
                        bias=bh_t[:, n : n + 1], accum_out=suma_b[:, n : n + 1],
                    )
                    ah2_0 = sc_pool.tile([128, MC], BF16, tag="ah2")
                    nc.scalar.activation(ah2_0[:], ah_t[:, 0:MC], ACTF.Square)
                    ah2_1 = sc_pool.tile([128, MC], BF16, tag="ah2")
                    nc.scalar.activation(ah2_1[:], ah_t[:, MC:NLOC], ACTF.Square)
                    # residual: yp = Ah + A  (kept in SBUF, bf16)
                    nc.vector.tensor_tensor(
                        YP[:, n * NLOC : (n + 1) * NLOC], ah_t[:],
                        A[:, n * NLOC : (n + 1) * NLOC], op=ALU.add,
                    )
                    prev_sq = (ah2_0, ah2_1, n)
                # last tile's sq matmuls
                pa0, pa1, pn = prev_sq
                nc.tensor.matmul(sqp0[:], ones_bf[:], pa0[:], start=False, stop=True)
                nc.tensor.matmul(sqp1[:], ones_bf[:], pa1[:], start=False, stop=True)

                # ---- stats + centered pre-BN write-back ----
                nc.vector.tensor_copy(sq_sb[:, 0:MC], sqp0[:])
                nc.vector.tensor_copy(sq_sb[:, MC:NLOC], sqp1[:])
                nc.vector.reduce_sum(ssq[:], sq_sb[:], axis=AX)
                nc.vector.tensor_scalar(
                    mhat[:], ssq[:], scalar1=1.0 / NLOC, scalar2=None, op0=ALU.mult,
                )
                nc.vector.tensor_scalar(
                    sq_c[:], sq_sb[:], scalar1=mhat[0:1, 0:1], scalar2=None,
                    op0=ALU.subtract,
                )
                nc.vector.tensor_copy(sq_cbf[:], sq_c[:])
                nc.gpsimd.partition_broadcast(bsq_bf[:], sq_cbf[:])
                nc.gpsimd.partition_broadcast(ssq_bc[:], ssq[:])
                nc.gpsimd.partition_broadcast(mhat_bc[:], mhat[:])

                # A <- y' = yp + (sq - mhat); sumY2 accumulated per h-tile
                for n in range(HT):
                    a_sl = A[:, n * NLOC : (n + 1) * NLOC]
                    nc.vector.tensor_tensor(
                        a_sl, YP[:, n * NLOC : (n + 1) * NLOC], bsq_bf[:], op=ALU.add,
                    )
                    scr = scr_pool.tile([128, NLOC], BF16, tag="scr")
                    nc.scalar.activation(
                        scr[:], a_sl, ACTF.Square, accum_out=sumY2[:, n : n + 1],
                    )

                # local shard stats:
                #   mu_s = (sumAh + sumA + ssq) / NLOC        (mean of true y)
                #   d    = mu_s - mhat                        (mean of y')
                #   Q_s  = E[y'^2] - d^2 + mu_s^2             (= E_shard[y^2])
                nc.vector.tensor_tensor(tmp1[:], suma_a[:], suma_b[:], op=ALU.add)
                nc.vector.tensor_tensor(tmp1[:], tmp1[:], sumA[:], op=ALU.add)
                nc.vector.tensor_scalar(
                    mu_s[:], tmp1[:], scalar1=ssq_bc[:, 0:1], scalar2=1.0 / NLOC,
                    op0=ALU.add, op1=ALU.mult,
                )
                nc.vector.tensor_scalar(
                    d_t[:], mu_s[:], scalar1=mhat_bc[:, 0:1], scalar2=None,
                    op0=ALU.subtract,
                )
                nc.vector.tensor_scalar(
                    q_t[:], sumY2[:], scalar1=1.0 / NLOC, scalar2=None, op0=ALU.mult,
                )
                nc.vector.tensor_tensor(tmp2[:], mu_s[:], d_t[:], op=ALU.subtract)
                nc.vector.tensor_tensor(tmp3[:], mu_s[:], d_t[:], op=ALU.add)
                nc.vector.tensor_tensor(tmp2[:], tmp2[:], tmp3[:], op=ALU.mult)
                nc.vector.tensor_tensor(q_t[:], q_t[:], tmp2[:], op=ALU.add)

                # AllReduce (mu_s, Q_s)
                cc_in2 = dpool.tile([128, 2 * HT], F32, tag="cc_in")
                cc_out2 = dpool.tile([128, 2 * HT], F32, tag="cc_out")
                nc.gpsimd.dma_start(cc_in2[:, 0:HT], mu_s[:])
                nc.gpsimd.dma_start(cc_in2[:, HT : 2 * HT], q_t[:])
                nc.gpsimd.collective_compute(
                    "AllReduce", ALU.add, replica_groups=groups,
                    ins=[cc_in2.opt()], outs=[cc_out2.opt()],
                )
                red = st2_pool.tile([128, 2 * HT], F32, tag="red")
                nc.gpsimd.dma_start(red[:], cc_out2[:])

                # global: mu = sum(mu_s)/8 ; var = sum(Q_s)/8 - mu^2
                mu = tmp1
                nc.vector.tensor_scalar(
                    mu[:], red[:, 0:HT], scalar1=1.0 / N_CORES, scalar2=None,
                    op0=ALU.mult,
                )
                nc.vector.tensor_scalar(
                    var_t[:], red[:, HT : 2 * HT], scalar1=1.0 / N_CORES,
                    scalar2=None, op0=ALU.mult,
                )
                nc.vector.tensor_tensor(tmp2[:], mu[:], mu[:], op=ALU.mult)
                nc.vector.tensor_tensor(var_t[:], var_t[:], tmp2[:], op=ALU.subtract)
                nc.scalar.activation(std_t[:], var_t[:], ACTF.Sqrt, bias=eps_t[:, 0:1])
                nc.vector.reciprocal(s_t[:], std_t[:])
                # c = s*(mhat - mu);  A_next = s*y' + c
                nc.vector.tensor_scalar(
                    tmp2[:], mu[:], scalar1=mhat_bc[:, 0:1], scalar2=None,
                    op0=ALU.subtract,
                )
                nc.vector.tensor_tensor(tmp2[:], tmp2[:], s_t[:], op=ALU.mult)
                nc.vector.tensor_scalar(
                    c_t[:], tmp2[:], scalar1=-1.0, scalar2=None, op0=ALU.mult,
                )
                # sum_m of next layer's (affine'd) A = NLOC*(s*d + c)
                nc.vector.tensor_tensor(tmp2[:], s_t[:], d_t[:], op=ALU.mult)
                nc.vector.tensor_tensor(tmp2[:], tmp2[:], c_t[:], op=ALU.add)
                nc.vector.tensor_scalar(
                    sumA[:], tmp2[:], scalar1=float(NLOC), scalar2=None, op0=ALU.mult,
                )

            # ---------------- last layer + final BN ----------------
            wl_t = st_pool.tile([128, HT * KIO], BF16)
            nc.sync.dma_start(wl_t[:], wlt_d[:, :])
            bl_t = st_pool.tile([KIO, 1], F32)
            nc.sync.dma_start(bl_t[:], blt_d[:, :])
            psl0 = ps_pool.tile([KIO, MC], F32, tag="ps0")
            psl1 = ps_pool.tile([KIO, MC], F32, tag="ps1")
            for k in range(HT):
                a_k = A[:, k * NLOC : (k + 1) * NLOC]
                nc.vector.tensor_scalar(
                    a_k, a_k, scalar1=s_t[:, k : k + 1], scalar2=c_t[:, k : k + 1],
                    op0=ALU.mult, op1=ALU.add,
                )
                lhsT = wl_t[:, k * KIO : (k + 1) * KIO]
                nc.tensor.matmul(
                    psl0[:], lhsT, a_k[:, 0:MC], start=(k == 0), stop=(k == HT - 1),
                )
                nc.tensor.matmul(
                    psl1[:], lhsT, a_k[:, MC:NLOC], start=(k == 0), stop=(k == HT - 1),
                )
            # yl = psl + b_last + X_in^T ; accumulate sums on the fly
            yl = st_pool.tile([KIO, NLOC], F32)
            syl_a = st_pool.tile([KIO, 1], F32)
            syl_b = st_pool.tile([KIO, 1], F32)
            nc.vector.scalar_tensor_tensor(
                yl[:, 0:MC], psl0[:], bl_t[:, 0:1], xt_f[:, 0:MC],
                op0=ALU.add, op1=ALU.add, accum_out=syl_a[:],
            )
            nc.vector.scalar_tensor_tensor(
                yl[:, MC:NLOC], psl1[:], bl_t[:, 0:1], xt_f[:, MC:NLOC],
                op0=ALU.add, op1=ALU.add, accum_out=syl_b[:],
            )
            scr2 = st_pool.tile([KIO, NLOC], F32)
            sy2l = st_pool.tile([KIO, 1], F32)
            nc.scalar.activation(scr2[:], yl[:], ACTF.Square, accum_out=sy2l[:])
            mu_l = st_pool.tile([KIO, 1], F32)
            nc.vector.tensor_tensor(mu_l[:], syl_a[:], syl_b[:], op=ALU.add)
            nc.vector.tensor_scalar(
                mu_l[:], mu_l[:], scalar1=1.0 / NLOC, scalar2=None, op0=ALU.mult,
            )
            q_l = st_pool.tile([KIO, 1], F32)
            nc.vector.tensor_scalar(
                q_l[:], sy2l[:], scalar1=1.0 / NLOC, scalar2=None, op0=ALU.mult,
            )
            cpl = st_pool.tile([KIO, 2], F32)
            nc.vector.tensor_copy(cpl[:, 0:1], mu_l[:])
            nc.vector.tensor_copy(cpl[:, 1:2], q_l[:])
            ccl_in = dpool.tile([KIO, 2], F32, tag="ccl_in")
            ccl_out = dpool.tile([KIO, 2], F32, tag="ccl_out")
            nc.gpsimd.dma_start(ccl_in[:], cpl[:])
            nc.gpsimd.collective_compute(
                "AllReduce", ALU.add, replica_groups=groups,
                ins=[ccl_in.opt()], outs=[ccl_out.opt()],
            )
            redl = st_pool.tile([KIO, 2], F32)
            nc.gpsimd.dma_start(redl[:], ccl_out[:])
            mugl = st_pool.tile([KIO, 1], F32)
            nc.vector.tensor_scalar(
                mugl[:], redl[:, 0:1], scalar1=1.0 / N_CORES, scalar2=None,
                op0=ALU.mult,
            )
            varl = st_pool.tile([KIO, 1], F32)
            nc.vector.tensor_scalar(
                varl[:], redl[:, 1:2], scalar1=1.0 / N_CORES, scalar2=None,
                op0=ALU.mult,
            )
            tl2 = st_pool.tile([KIO, 1], F32)
            nc.vector.tensor_tensor(tl2[:], mugl[:], mugl[:], op=ALU.mult)
            nc.vector.tensor_tensor(varl[:], varl[:], tl2[:], op=ALU.subtract)
            stdl = st_pool.tile([KIO, 1], F32)
            nc.scalar.activation(stdl[:], varl[:], ACTF.Sqrt, bias=eps_t[0:KIO, 0:1])
            sl_t = st_pool.tile([KIO, 1], F32)
            nc.vector.reciprocal(sl_t[:], stdl[:])
            cl_t = st_pool.tile([KIO, 1], F32)
            nc.vector.tensor_tensor(cl_t[:], mugl[:], sl_t[:], op=ALU.mult)
            nc.vector.tensor_scalar(
                cl_t[:], cl_t[:], scalar1=-1.0, scalar2=None, op0=ALU.mult,
            )
            yout = st_pool.tile([KIO, NLOC], F32)
            nc.vector.tensor_scalar(
                yout[:], yl[:], scalar1=sl_t[:, 0:1], scalar2=cl_t[:, 0:1],
                op0=ALU.mult, op1=ALU.add,
            )
            nc.sync.dma_start(y_d[:, :], yout[:])

    nc.compile()
    return nc


def _get_nc():
    if "nc" not in _CACHE:
        _CACHE["nc"] = _build()
    return _CACHE["nc"]


def make_in_maps(inputs):
    """Host-side prep: shard X over cores, pre-permute/cast weights."""
    import ml_dtypes

    bf16 = ml_dtypes.bfloat16
    x = np.asarray(inputs["X_in"], np.float32)
    wf = np.asarray(inputs["W_first"], np.float32)
    bf = np.asarray(inputs["b_first"], np.float32)
    wh = np.asarray(inputs["W_h"], np.float32)
    bh = np.asarray(inputs["b_h"], np.float32)
    wl = np.asarray(inputs["W_last"], np.float32)
    bl = np.asarray(inputs["b_last"], np.float32)

    # whp[d, n, p, t, c] = W_h[d, t*128+p, n*128+c]
    whp = np.ascontiguousarray(
        wh.reshape(D, HT, 128, HT, 128).transpose(0, 3, 2, 1, 4)
    ).astype(bf16)
    bht = np.ascontiguousarray(bh.reshape(D, HT, 128).transpose(0, 2, 1))
    bft = np.ascontiguousarray(bf.reshape(HT, 128).T)
    # wlt[p, t*KIO + c] = W_last[t*128+p, c]
    wlt = np.ascontiguousarray(
        wl.reshape(HT, 128, KIO).transpose(1, 0, 2).reshape(128, HT * KIO)
    ).astype(bf16)
    shared = {
        "wf": np.ascontiguousarray(wf).astype(bf16),
        "bft": bft,
        "whp": whp,
        "bht": bht,
        "wlt": wlt,
        "blt": np.ascontiguousarray(bl.reshape(KIO, 1)),
    }
    in_maps = []
    for c in range(N_CORES):
        xs = np.ascontiguousarray(x[c * NLOC : (c + 1) * NLOC].T)  # [KIO, NLOC]
        in_maps.append(
            {"xt_bf": xs.astype(bf16), "xt_f": xs, **shared}
        )
    return in_maps


def kernel(**inputs):
    from concourse.bass_utils import run_bass_kernel_spmd

    nc = _get_nc()
    in_maps = make_in_maps(inputs)
    res = run_bass_kernel_spmd(nc, in_maps, list(range(N_CORES)))
    out = np.concatenate(
        [res.results[c]["y"].T for c in range(N_CORES)], axis=0
    )
    return np.ascontiguousarray(out.astype(np.float32))


# revision 3
# speedup vs baseline: 1.3587x; 1.0306x over previous
"""Trainium2 Bass kernel for nn_DeepNet (dense MLP with BatchNorm over batch).

Reference computation (N=8192 rows, K=2 in/out features, H=4096 hidden, D=3):
    X = relu(X_in @ W_first + b_first)                      # [N, H]
    for i in range(3):
        Xh = relu(X @ W_h[i] + b_h[i])                      # [N, H]
        sq = rowwise_sum(Xh * Xh)                           # [N, 1]
        X  = bn(sq + Xh + X)        # batch stats over N, per hidden unit
    out = bn(X @ W_last + b_last + X_in)                    # [N, 2]

Strategy: data-parallel over N across 8 NeuronCores (1024 rows/core).
Activations live in SBUF transposed: A[h, m] = X[m, h].

v2 design vs baseline:
  - Weights + activations in bf16: LDWEIGHTS drops 224->~107ns (fully hidden
    behind the 213ns N=512 matmul stream); fp32r paid 272ns/MM = LDW-bound.
  - Host-side prep: weights pre-permuted to DMA-contiguous layout + cast to
    bf16; X_in pre-transposed; output returned transposed (host transposes
    back). Kills the 9us/element-descriptor DMAs at head/tail.
  - No DRAM spill: yp = Ah + A kept in a second SBUF buffer (bf16).
  - One-pass BN stats: A <- y' = yp + (sq - mhat) (centered pre-BN, bf16);
    var via E[y'^2] with the Welford-style shift identity Q_s = E_shard[y^2]
    = E[y'^2] - d^2 + mu_s^2 (no catastrophic cancellation; only the final
    global var = mean(Q) - mu^2 subtracts big numbers, err ~0.04% of var).
  - AllReduce payload (mu_s, Q_s) = 32KB, one per hidden layer + tiny final.
  - BN affine (scale/shift) folded into the *next* layer's n=0 k-loop as an
    in-place tensor_scalar per k-tile, pipelined under the matmuls.
  - sq ones-matmuls delayed by one n-tile so PE never waits on Scalar.
"""

import numpy as np

N_CORES = 8
N = 8192
NLOC = N // N_CORES  # 1024 rows per core
KIO = 2
H = 4096
HT = H // 128  # 32 hidden-dim tiles
D = 3
MC = 512  # matmul moving-operand chunk (one PSUM bank of fp32)
EPS = 1e-5

_CACHE = {}


def _build():
    import concourse.bass as bass  # noqa: F401  (registers engines)
    import concourse.mybir as mybir
    import concourse.tile as tile
    from concourse import bacc

    F32 = mybir.dt.float32
    BF16 = mybir.dt.bfloat16
    ALU = mybir.AluOpType
    ACTF = mybir.ActivationFunctionType
    AX = mybir.AxisListType.X

    nc = bacc.Bacc("TRN2", target_bir_lowering=False, debug=False, num_devices=N_CORES)

    F8 = mybir.dt.float8e4
    F32R = mybir.dt.float32r
    DR = mybir.MatmulPerfMode.DoubleRow

    xt_bf_d = nc.dram_tensor("xt_bf", [KIO, NLOC], BF16, kind="ExternalInput")
    xt_f_d = nc.dram_tensor("xt_f", [KIO, NLOC], F32, kind="ExternalInput")
    wf_d = nc.dram_tensor("wf", [KIO, H], BF16, kind="ExternalInput")
    bft_d = nc.dram_tensor("bft", [128, HT], F32, kind="ExternalInput")
    # layer 0 hidden weights in bf16 (activations there are full-magnitude,
    # fp8 residual trick doesn't apply)
    whp_d = nc.dram_tensor("whp", [HT, 128, HT, 128], BF16, kind="ExternalInput")
    # layers 1,2: fp8 DoubleRow-packed weights (x2048) + bf16 natural-layout
    # shard for the u = W^T s matvec
    whp8_d = nc.dram_tensor(
        "whp8", [D - 1, HT, 128, HT // 2, 2, 128], F8, kind="ExternalInput"
    )
    wnat_d = nc.dram_tensor(
        "wnat", [D - 1, 128, HT * (H // N_CORES)], BF16, kind="ExternalInput"
    )
    bht_d = nc.dram_tensor("bht", [D, 128, HT], F32, kind="ExternalInput")
    wlt_d = nc.dram_tensor("wlt", [128, HT * KIO], BF16, kind="ExternalInput")
    blt_d = nc.dram_tensor("blt", [KIO, 1], F32, kind="ExternalInput")
    y_d = nc.dram_tensor("y", [KIO, NLOC], F32, kind="ExternalOutput")

    SW = 2048.0  # fp8 weight scale
    SA = 256.0   # fp8 activation-residual scale
    DESCALE = 1.0 / (SW * SA)

    groups = [list(range(N_CORES))]

    with tile.TileContext(nc) as tc:
        with (
            tc.tile_pool(name="a", bufs=1) as apool,
            tc.tile_pool(name="yp", bufs=1) as yppool,
            tc.tile_pool(name="w8", bufs=2) as w8pool,
            tc.tile_pool(name="big8", bufs=1) as big8pool,
            tc.tile_pool(name="un", bufs=3) as un_pool,
            tc.tile_pool(name="ahw", bufs=3) as ahw_pool,
            tc.tile_pool(name="sc", bufs=4) as sc_pool,
            tc.tile_pool(name="st", bufs=1) as st_pool,
            tc.tile_pool(name="st2", bufs=2) as st2_pool,
            tc.tile_pool(name="ps", bufs=2, space="PSUM") as ps_pool,
            tc.tile_pool(name="sqps", bufs=1, space="PSUM") as sq_pool,
            tc.tile_pool(name="dram", bufs=1, space="DRAM") as dpool,
        ):
            A = apool.tile([128, HT * NLOC], BF16)
            YP = yppool.tile([128, HT * NLOC], BF16)
            # big8 is time-multiplexed scratch (32KB/partition):
            #   t0: L1 weight/input staging + layer-0 bf16 W double-buffer
            #   t1: W-natural bf16 shard for the u matvec (per DR layer)
            #   t2: fp8 residual activations A8 (DR matmul phase)
            #   t3: last-layer f32 scratch
            big8 = big8pool.tile([128, HT * MC], BF16)

            ones_bf = st_pool.tile([128, 1], BF16)
            nc.vector.memset(ones_bf[:], 1.0)
            ones_f = st_pool.tile([128, 1], F32)
            nc.vector.memset(ones_f[:], 1.0)
            eps_t = st_pool.tile([128, 1], F32)
            nc.vector.memset(eps_t[:], EPS)

            xt_bf = big8[0:KIO, 12288:12288 + NLOC]
            nc.sync.dma_start(xt_bf, xt_bf_d[:, :])
            xt_f = st_pool.tile([KIO, NLOC], F32)
            nc.sync.dma_start(xt_f[:], xt_f_d[:, :])
            wf_t = big8[0:KIO, 8192:8192 + H]
            nc.sync.dma_start(wf_t, wf_d[:, :])
            bf_t = st_pool.tile([128, HT], F32)
            nc.sync.dma_start(bf_t[:], bft_d[:, :])

            # per-h stat accumulators (f32)
            suma_a = st_pool.tile([128, HT], F32)
            suma_b = st_pool.tile([128, HT], F32)
            sumA = st_pool.tile([128, HT], F32)
            sumY2 = st_pool.tile([128, HT], F32)
            mu_s = st_pool.tile([128, HT], F32)
            d_t = st_pool.tile([128, HT], F32)
            q_t = st_pool.tile([128, HT], F32)
            tmp1 = st_pool.tile([128, HT], F32)
            tmp2 = st_pool.tile([128, HT], F32)
            tmp3 = st_pool.tile([128, HT], F32)
            var_t = st_pool.tile([128, HT], F32)
            std_t = st_pool.tile([128, HT], F32)
            s_t = st_pool.tile([128, HT], F32)
            c_t = st_pool.tile([128, HT], F32)
            sq_sb = st_pool.tile([1, NLOC], F32)
            sq_c = st_pool.tile([1, NLOC], F32)
            sq_cbf = st_pool.tile([1, NLOC], BF16)
            bsq_bf = st_pool.tile([128, NLOC], BF16)
            ssq = st_pool.tile([1, 1], F32)
            mhat = st_pool.tile([1, 1], F32)
            ssq_bc = st_pool.tile([128, 1], F32)
            mhat_bc = st_pool.tile([128, 1], F32)

            # ---------------- first layer: A = relu(W_first^T X^T + b) -------
            for n in range(HT):
                ps0 = ps_pool.tile([128, MC], F32, tag="ps0")
                ps1 = ps_pool.tile([128, MC], F32, tag="ps1")
                lhsT = wf_t[:, n * 128 : (n + 1) * 128]
                nc.tensor.matmul(ps0[:], lhsT, xt_bf[:, 0:MC], start=True, stop=True)
                nc.tensor.matmul(ps1[:], lhsT, xt_bf[:, MC:NLOC], start=True, stop=True)
                a_sl = A[:, n * NLOC : (n + 1) * NLOC]
                nc.scalar.activation(
                    a_sl[:, 0:MC], ps0[:], ACTF.Relu,
                    bias=bf_t[:, n : n + 1], accum_out=suma_a[:, n : n + 1],
                )
                nc.scalar.activation(
                    a_sl[:, MC:NLOC], ps1[:], ACTF.Relu,
                    bias=bf_t[:, n : n + 1], accum_out=suma_b[:, n : n + 1],
                )
            nc.vector.tensor_tensor(sumA[:], suma_a[:], suma_b[:], op=ALU.add)

            # warm up the collective path with a full-size AllReduce while the
            # PE is busy with layer 1 (cold first AR otherwise costs ~2x)
            cc_in = dpool.tile([128, 2 * HT], F32, tag="cc_in")
            cc_out = dpool.tile([128, 2 * HT], F32, tag="cc_out")
            nc.gpsimd.dma_start(cc_in[:, 0:HT], suma_a[:])
            nc.gpsimd.dma_start(cc_in[:, HT : 2 * HT], suma_b[:])
            nc.gpsimd.collective_compute(
                "AllReduce", ALU.add, replica_groups=groups,
                ins=[cc_in.opt()], outs=[cc_out.opt()],
            )

            # ---------------- hidden layers ----------------
            s_bf = st_pool.tile([128, HT], BF16)
            s8_t = st_pool.tile([128, HT], F32)
            c8_t = st_pool.tile([128, HT], F32)
            bsq2 = st_pool.tile([1, NLOC], F32R)
            A8r = big8[:].bitcast(F8).rearrange("p (kt m) -> p kt m", m=NLOC)

            for li in range(D):
                dr = li > 0
                bh_t = st2_pool.tile([128, HT], F32, tag="bh")
                nc.sync.dma_start(bh_t[:], bht_d[li])
                sqp0 = sq_pool.tile([1, MC], F32, tag="sq0")
                sqp1 = sq_pool.tile([1, MC], F32, tag="sq1")

                if dr:
                    # u = W^T s on own n'-shard (reads wnat staged in big8),
                    # then AllGather; scale folds the fp8 descale
                    nc.vector.tensor_copy(s_bf[:], s_t[:])
                    usm = sq_pool.tile([1, MC], F32, tag="usm")
                    for kt in range(HT):
                        nc.tensor.matmul(
                            usm[:], s_bf[:, kt : kt + 1],
                            big8[:, kt * MC : (kt + 1) * MC],
                            start=(kt == 0), stop=(kt == HT - 1),
                        )
                    u_sb = st_pool.tile([1, MC], F32)
                    nc.vector.tensor_scalar(
                        u_sb[:], usm[:], scalar1=SW * SA, scalar2=None, op0=ALU.mult,
                    )
                    uag_in = dpool.tile([1, MC], F32, tag="uag_in")
                    uag_out = dpool.tile([N_CORES, MC], F32, tag="uag_out")
                    nc.gpsimd.dma_start(uag_in[:], u_sb[:])
                    nc.gpsimd.collective_compute(
                        "AllGather", ALU.bypass, replica_groups=groups,
                        ins=[uag_in.opt()], outs=[uag_out.opt()],
                    )
                    uag_flat = uag_out[:].rearrange("r m -> (r m)").unsqueeze(0)
                    nc.vector.tensor_scalar(
                        s8_t[:], s_t[:], scalar1=SA, scalar2=None, op0=ALU.mult,
                    )
                    nc.vector.tensor_scalar(
                        c8_t[:], c_t[:], scalar1=SA, scalar2=None, op0=ALU.mult,
                    )
                    nc.vector.tensor_copy(bsq2[:], sq_c[:])

                prev_sq = None
                for n in range(HT):
                    ps0 = ps_pool.tile([128, MC], F32, tag="ps0")
                    ps1 = ps_pool.tile([128, MC], F32, tag="ps1")
                    if dr:
                        wcol8 = w8pool.tile([128, (HT // 2) * 256], F8, tag="w8")
                        nc.sync.dma_start(
                            wcol8[:].rearrange("p (kp ko c) -> p kp ko c", ko=2, c=128),
                            whp8_d[li - 1, n],
                        )
                        un_t = un_pool.tile([1, 128], F32, tag="un")
                        nc.sync.dma_start(
                            un_t[:], uag_flat[0:1, n * 128 : (n + 1) * 128]
                        )
                        for kp in range(HT // 2):
                            if n == 0:
                                for j in range(2):
                                    kt = 2 * kp + j
                                    eng = nc.vector if j == 0 else nc.gpsimd
                                    eng.tensor_scalar(
                                        big8[:, kt * MC : (kt + 1) * MC].bitcast(F8),
                                        YP[:, kt * NLOC : (kt + 1) * NLOC],
                                        scalar1=s8_t[:, kt : kt + 1],
                                        scalar2=c8_t[:, kt : kt + 1],
                                        op0=ALU.mult, op1=ALU.add,
                                    )
                            lhsT = wcol8[:, kp * 256 : (kp + 1) * 256].rearrange(
                                "p (ko c) -> p ko c", ko=2
                            )
                            nc.tensor.matmul(
                                ps0[:], lhsT, A8r[:, 2 * kp : 2 * kp + 2, 0:MC],
                                start=(kp == 0), stop=False, perf_mode=DR,
                            )
                            nc.tensor.matmul(
                                ps1[:], lhsT, A8r[:, 2 * kp : 2 * kp + 2, MC:NLOC],
                                start=(kp == 0), stop=False, perf_mode=DR,
                            )
                        # rank-1 term u (x) (sq - mhat) closes the group
                        un = un_t[0:1, :].bitcast(F32R)
                        nc.tensor.matmul(
                            ps0[:], un, bsq2[:, 0:MC], start=False, stop=True,
                        )
                        nc.tensor.matmul(
                            ps1[:], un, bsq2[:, MC:NLOC], start=False, stop=True,
                        )
                        # affine y' -> X for this h-tile (residual add needs it)
                        a_n = A[:, n * NLOC : (n + 1) * NLOC]
                        nc.vector.tensor_scalar(
                            a_n, a_n, scalar1=s_t[:, n : n + 1],
                            scalar2=c_t[:, n : n + 1], op0=ALU.mult, op1=ALU.add,
                        )
                    else:
                        # layer-0 bf16 weights double-buffer in big8[:, 0:8192]
                        wcol = big8[:, (n % 2) * 4096 : (n % 2) * 4096 + 4096]
                        nc.sync.dma_start(
                            wcol.rearrange("p (t c) -> p t c", c=128),
                            whp_d[n],
                        )
                        for k in range(HT):
                            a_k = A[:, k * NLOC : (k + 1) * NLOC]
                            lhsT = wcol[:, k * 128 : (k + 1) * 128]
                            nc.tensor.matmul(
                                ps0[:], lhsT, a_k[:, 0:MC],
                                start=(k == 0), stop=(k == HT - 1),
                            )
                            nc.tensor.matmul(
                                ps1[:], lhsT, a_k[:, MC:NLOC],
                                start=(k == 0), stop=(k == HT - 1),
                            )
                    # sq matmuls for the PREVIOUS tile (gives Scalar a full
                    # k-loop of slack -> PE never waits on relu/square)
                    if prev_sq is not None:
                        pa0, pa1, pn = prev_sq
                        nc.tensor.matmul(
                            sqp0[:], ones_bf[:], pa0[:],
                            start=(pn == 0), stop=False,
                        )
                        nc.tensor.matmul(
                            sqp1[:], ones_bf[:], pa1[:],
                            start=(pn == 0), stop=False,
                        )
                    ah_t = ahw_pool.tile([128, NLOC], BF16, tag="ahw")
                    descale = DESCALE if dr else 1.0
                    nc.scalar.activation(
                        ah_t[:, 0:MC], ps0[:], ACTF.Relu, scale=descale,
                        bias=bh_t[:, n : n + 1], accum_out=suma_a[:, n : n + 1],
                    )
                    nc.scalar.activation(
                        ah_t[:, MC:NLOC], ps1[:], ACTF.Relu, scale=descale,
                        bias=bh_t[:, n : n + 1], accum_out=suma_b[:, n : n + 1],
                    )
                    ah2_0 = sc_pool.tile([128, MC], BF16, tag="ah2")
                    nc.scalar.activation(ah2_0[:], ah_t[:, 0:MC], ACTF.Square)
                    ah2_1 = sc_pool.tile([128, MC], BF16, tag="ah2")
                    nc.scalar.activation(ah2_1[:], ah_t[:, MC:NLOC], ACTF.Square)
                    # residual: yp = Ah + A  (kept in SBUF, bf16)
                    nc.vector.tensor_tensor(
                        YP[:, n * NLOC : (n + 1) * NLOC], ah_t[:],
                        A[:, n * NLOC : (n + 1) * NLOC], op=ALU.add,
                    )
                    prev_sq = (ah2_0, ah2_1, n)
                # last tile's sq matmuls
                pa0, pa1, pn = prev_sq
                nc.tensor.matmul(sqp0[:], ones_bf[:], pa0[:], start=False, stop=True)
                nc.tensor.matmul(sqp1[:], ones_bf[:], pa1[:], start=False, stop=True)

                # ---- stats + centered pre-BN write-back ----
                nc.vector.tensor_copy(sq_sb[:, 0:MC], sqp0[:])
                nc.vector.tensor_copy(sq_sb[:, MC:NLOC], sqp1[:])
                nc.vector.reduce_sum(ssq[:], sq_sb[:], axis=AX)
                nc.vector.tensor_scalar(
                    mhat[:], ssq[:], scalar1=1.0 / NLOC, scalar2=None, op0=ALU.mult,
                )
                nc.vector.tensor_scalar(
                    sq_c[:], sq_sb[:], scalar1=mhat[0:1, 0:1], scalar2=None,
                    op0=ALU.subtract,
                )
                nc.vector.tensor_copy(sq_cbf[:], sq_c[:])
                nc.gpsimd.partition_broadcast(bsq_bf[:], sq_cbf[:])
                nc.gpsimd.partition_broadcast(ssq_bc[:], ssq[:])
                nc.gpsimd.partition_broadcast(mhat_bc[:], mhat[:])
                if li < D - 1:
                    # stage next layer's W-natural shard for the u matvec
                    # (overlaps the stats pass; big8's fp8 alias was fully
                    # consumed by this layer's matmul phase)
                    nc.sync.dma_start(big8[:], wnat_d[li])

                # A <- y' = yp + (sq - mhat); sumY2 accumulated per h-tile.
                # Square+accum is split DVE/Scalar (Scalar alone is the
                # bottleneck at ~1.4us/tile); periodic tiny matmuls keep the
                # PE's HAM clock-gate warm through this PE-idle stretch.
                for n in range(HT):
                    a_sl = A[:, n * NLOC : (n + 1) * NLOC]
                    nc.vector.tensor_tensor(
                        a_sl, YP[:, n * NLOC : (n + 1) * NLOC], bsq_bf[:], op=ALU.add,
                    )
                    scr = ahw_pool.tile([128, NLOC], BF16, tag="ahw")
                    nc.scalar.activation(
                        scr[:], a_sl, ACTF.Square, accum_out=sumY2[:, n : n + 1],
                    )

                # local shard stats:
                #   mu_s = (sumAh + sumA + ssq) / NLOC        (mean of true y)
                #   d    = mu_s - mhat                        (mean of y')
                #   Q_s  = E[y'^2] - d^2 + mu_s^2             (= E_shard[y^2])
                nc.vector.tensor_tensor(tmp1[:], suma_a[:], suma_b[:], op=ALU.add)
                nc.vector.tensor_tensor(tmp1[:], tmp1[:], sumA[:], op=ALU.add)
                nc.vector.tensor_scalar(
                    mu_s[:], tmp1[:], scalar1=ssq_bc[:, 0:1], scalar2=1.0 / NLOC,
                    op0=ALU.add, op1=ALU.mult,
                )
                nc.vector.tensor_scalar(
                    d_t[:], mu_s[:], scalar1=mhat_bc[:, 0:1], scalar2=None,
                    op0=ALU.subtract,
                )
                nc.vector.tensor_scalar(
                    q_t[:], sumY2[:], scalar1=1.0 / NLOC, scalar2=None, op0=ALU.mult,
                )
                nc.vector.tensor_tensor(tmp2[:], mu_s[:], d_t[:], op=ALU.subtract)
                nc.vector.tensor_tensor(tmp3[:], mu_s[:], d_t[:], op=ALU.add)
                nc.vector.tensor_tensor(tmp2[:], tmp2[:], tmp3[:], op=ALU.mult)
                nc.vector.tensor_tensor(q_t[:], q_t[:], tmp2[:], op=ALU.add)

                # AllReduce (mu_s, Q_s)
                cc_in2 = dpool.tile([128, 2 * HT], F32, tag="cc_in")
                cc_out2 = dpool.tile([128, 2 * HT], F32, tag="cc_out")
                nc.gpsimd.dma_start(cc_in2[:, 0:HT], mu_s[:])
                nc.gpsimd.dma_start(cc_in2[:, HT : 2 * HT], q_t[:])
                nc.gpsimd.collective_compute(
                    "AllReduce", ALU.add, replica_groups=groups,
                    ins=[cc_in2.opt()], outs=[cc_out2.opt()],
                )
                red = st2_pool.tile([128, 2 * HT], F32, tag="red")
                nc.gpsimd.dma_start(red[:], cc_out2[:])

                # global: mu = sum(mu_s)/8 ; var = sum(Q_s)/8 - mu^2
                mu = tmp1
                nc.vector.tensor_scalar(
                    mu[:], red[:, 0:HT], scalar1=1.0 / N_CORES, scalar2=None,
                    op0=ALU.mult,
                )
                nc.vector.tensor_scalar(
                    var_t[:], red[:, HT : 2 * HT], scalar1=1.0 / N_CORES,
                    scalar2=None, op0=ALU.mult,
                )
                nc.vector.tensor_tensor(tmp2[:], mu[:], mu[:], op=ALU.mult)
                nc.vector.tensor_tensor(var_t[:], var_t[:], tmp2[:], op=ALU.subtract)
                nc.scalar.activation(std_t[:], var_t[:], ACTF.Sqrt, bias=eps_t[:, 0:1])
                nc.vector.reciprocal(s_t[:], std_t[:])
                # c = s*(mhat - mu);  A_next = s*y' + c
                nc.vector.tensor_scalar(
                    tmp2[:], mu[:], scalar1=mhat_bc[:, 0:1], scalar2=None,
                    op0=ALU.subtract,
                )
                nc.vector.tensor_tensor(tmp2[:], tmp2[:], s_t[:], op=ALU.mult)
                nc.vector.tensor_scalar(
                    c_t[:], tmp2[:], scalar1=-1.0, scalar2=None, op0=ALU.mult,
                )
                # sum_m of next layer's (affine'd) A = NLOC*(s*d + c)
                nc.vector.tensor_tensor(tmp2[:], s_t[:], d_t[:], op=ALU.mult)
                nc.vector.tensor_tensor(tmp2[:], tmp2[:], c_t[:], op=ALU.add)
                nc.vector.tensor_scalar(
                    sumA[:], tmp2[:], scalar1=float(NLOC), scalar2=None, op0=ALU.mult,
                )

            # ---------------- last layer + final BN ----------------
            wl_t = st_pool.tile([128, HT * KIO], BF16)
            nc.sync.dma_start(wl_t[:], wlt_d[:, :])
            bl_t = st_pool.tile([KIO, 1], F32)
            nc.sync.dma_start(bl_t[:], blt_d[:, :])
            psl0 = ps_pool.tile([KIO, MC], F32, tag="ps0")
            psl1 = ps_pool.tile([KIO, MC], F32, tag="ps1")
            for k in range(HT):
                a_k = A[:, k * NLOC : (k + 1) * NLOC]
                nc.vector.tensor_scalar(
                    a_k, a_k, scalar1=s_t[:, k : k + 1], scalar2=c_t[:, k : k + 1],
                    op0=ALU.mult, op1=ALU.add,
                )
                lhsT = wl_t[:, k * KIO : (k + 1) * KIO]
                nc.tensor.matmul(
                    psl0[:], lhsT, a_k[:, 0:MC], start=(k == 0), stop=(k == HT - 1),
                )
                nc.tensor.matmul(
                    psl1[:], lhsT, a_k[:, MC:NLOC], start=(k == 0), stop=(k == HT - 1),
                )
            # yl = psl + b_last + X_in^T ; accumulate sums on the fly
            yl = big8[0:KIO, 0:2048].bitcast(F32)
            syl_a = st_pool.tile([KIO, 1], F32)
            syl_b = st_pool.tile([KIO, 1], F32)
            nc.vector.scalar_tensor_tensor(
                yl[:, 0:MC], psl0[:], bl_t[:, 0:1], xt_f[:, 0:MC],
                op0=ALU.add, op1=ALU.add, accum_out=syl_a[:],
            )
            nc.vector.scalar_tensor_tensor(
                yl[:, MC:NLOC], psl1[:], bl_t[:, 0:1], xt_f[:, MC:NLOC],
                op0=ALU.add, op1=ALU.add, accum_out=syl_b[:],
            )
            scr2 = big8[0:KIO, 2048:4096].bitcast(F32)
            sy2l = st_pool.tile([KIO, 1], F32)
            nc.scalar.activation(scr2[:], yl[:], ACTF.Square, accum_out=sy2l[:])
            mu_l = st_pool.tile([KIO, 1], F32)
            nc.vector.tensor_tensor(mu_l[:], syl_a[:], syl_b[:], op=ALU.add)
            nc.vector.tensor_scalar(
                mu_l[:], mu_l[:], scalar1=1.0 / NLOC, scalar2=None, op0=ALU.mult,
            )
            q_l = st_pool.tile([KIO, 1], F32)
            nc.vector.tensor_scalar(
                q_l[:], sy2l[:], scalar1=1.0 / NLOC, scalar2=None, op0=ALU.mult,
            )
            cpl = st_pool.tile([KIO, 2], F32)
            nc.vector.tensor_copy(cpl[:, 0:1], mu_l[:])
            nc.vector.tensor_copy(cpl[:, 1:2], q_l[:])
            ccl_in = dpool.tile([KIO, 2], F32, tag="ccl_in")
            ccl_out = dpool.tile([KIO, 2], F32, tag="ccl_out")
            nc.gpsimd.dma_start(ccl_in[:], cpl[:])
            nc.gpsimd.collective_compute(
                "AllReduce", ALU.add, replica_groups=groups,
                ins=[ccl_in.opt()], outs=[ccl_out.opt()],
            )
            redl = st_pool.tile([KIO, 2], F32)
            nc.gpsimd.dma_start(redl[:], ccl_out[:])
            mugl = st_pool.tile([KIO, 1], F32)
            nc.vector.tensor_scalar(
                mugl[:], redl[:, 0:1], scalar1=1.0 / N_CORES, scalar2=None,
                op0=ALU.mult,
            )
            varl = st_pool.tile([KIO, 1], F32)
            nc.vector.tensor_scalar(
                varl[:], redl[:, 1:2], scalar1=1.0 / N_CORES, scalar2=None,
                op0=ALU.mult,
            )
            tl2 = st_pool.tile([KIO, 1], F32)
            nc.vector.tensor_tensor(tl2[:], mugl[:], mugl[:], op=ALU.mult)
            nc.vector.tensor_tensor(varl[:], varl[:], tl2[:], op=ALU.subtract)
            stdl = st_pool.tile([KIO, 1], F32)
            nc.scalar.activation(stdl[:], varl[:], ACTF.Sqrt, bias=eps_t[0:KIO, 0:1])
            sl_t = st_pool.tile([KIO, 1], F32)
            nc.vector.reciprocal(sl_t[:], stdl[:])
            cl_t = st_pool.tile([KIO, 1], F32)
            nc.vector.tensor_tensor(cl_t[:], mugl[:], sl_t[:], op=ALU.mult)
            nc.vector.tensor_scalar(
                cl_t[:], cl_t[:], scalar1=-1.0, scalar2=None, op0=ALU.mult,
            )
            yout = big8[0:KIO, 4096:6144].bitcast(F32)
            nc.vector.tensor_scalar(
                yout[:], yl[:], scalar1=sl_t[:, 0:1], scalar2=cl_t[:, 0:1],
                op0=ALU.mult, op1=ALU.add,
            )
            nc.sync.dma_start(y_d[:, :], yout[:])

    nc.compile()
    return nc


def _get_nc():
    if "nc" not in _CACHE:
        _CACHE["nc"] = _build()
    return _CACHE["nc"]


def make_in_maps(inputs):
    """Host-side prep: shard X over cores, pre-permute/cast weights."""
    import ml_dtypes

    bf16 = ml_dtypes.bfloat16
    x = np.asarray(inputs["X_in"], np.float32)
    wf = np.asarray(inputs["W_first"], np.float32)
    bf = np.asarray(inputs["b_first"], np.float32)
    wh = np.asarray(inputs["W_h"], np.float32)
    bh = np.asarray(inputs["b_h"], np.float32)
    wl = np.asarray(inputs["W_last"], np.float32)
    bl = np.asarray(inputs["b_last"], np.float32)

    f8 = ml_dtypes.float8_e4m3
    SW, SA = 2048.0, 256.0
    # layer 0 (bf16): whp[n, p, t, c] = W_h[0, t*128+p, n*128+c]
    whp = np.ascontiguousarray(
        wh[0].reshape(HT, 128, HT, 128).transpose(2, 1, 0, 3)
    ).astype(bf16)
    # layers 1,2 fp8 DoubleRow pack:
    # whp8[d, n, p, kp, ko, c] = fp8(SW * W_h[d+1, kp*256+ko*128+p, n*128+c])
    w12 = wh[1:].reshape(2, HT // 2, 2, 128, HT, 128)  # [d, kp, ko, p, n, c]
    whp8 = np.ascontiguousarray(
        np.clip(w12 * SW, -240.0, 240.0).transpose(0, 4, 3, 1, 2, 5)
    ).astype(f8)
    bht = np.ascontiguousarray(bh.reshape(D, HT, 128).transpose(0, 2, 1))
    bft = np.ascontiguousarray(bf.reshape(HT, 128).T)
    # wlt[p, t*KIO + c] = W_last[t*128+p, c]
    wlt = np.ascontiguousarray(
        wl.reshape(HT, 128, KIO).transpose(1, 0, 2).reshape(128, HT * KIO)
    ).astype(bf16)
    shared = {
        "wf": np.ascontiguousarray(wf).astype(bf16),
        "bft": bft,
        "whp": whp,
        "whp8": whp8,
        "bht": bht,
        "wlt": wlt,
        "blt": np.ascontiguousarray(bl.reshape(KIO, 1)),
    }
    CS = H // N_CORES  # 512 output columns of u per core
    in_maps = []
    for c in range(N_CORES):
        xs = np.ascontiguousarray(x[c * NLOC : (c + 1) * NLOC].T)  # [KIO, NLOC]
        # wnat[d, p, kt, j] = W_h[d+1, kt*128+p, c*CS+j]  (per-core n'-shard)
        wnat = (
            np.ascontiguousarray(
                wh[1:, :, c * CS : (c + 1) * CS]
                .reshape(2, HT, 128, CS)
                .transpose(0, 2, 1, 3)
            )
            .astype(bf16)
            .reshape(2, 128, HT * CS)
        )
        in_maps.append(
            {"xt_bf": xs.astype(bf16), "xt_f": xs, "wnat": wnat, **shared}
        )
    return in_maps


def kernel(**inputs):
    from concourse.bass_utils import run_bass_kernel_spmd

    nc = _get_nc()
    in_maps = make_in_maps(inputs)
    res = run_bass_kernel_spmd(nc, in_maps, list(range(N_CORES)))
    out = np.concatenate(
        [res.results[c]["y"].T for c in range(N_CORES)], axis=0
    )
    return np.ascontiguousarray(out.astype(np.float32))
